# revision 13
# baseline (speedup 1.0000x reference)
"""Masked fractional Hamming distance over 31 circular rotations, on 8 trn2 cores.

Math: for shift s, num(s)/den(s) with
  den(s) = sum maskbits, num(s) = masked differing bits.
Encode A = (ia<<7)|ma, B = (ib<<7)|mb on the host; read as fp8e4m3 the bytes
become {+0, -0, +2^-9, -2^-9} (sign=iris, magnitude=mask), so
  corr(A, B)(s)       = (den - 2*num) * 2^-18
  corr(A&7F, B&7F)(s) = den * 2^-18       (mask strip done on-device)
Only A and B ship to the device (half the HBM traffic of raw ia/ma/ib/mb).

The fused (l,k) code axis only ever shifts by even amounts, so the host
deinterleaves it into two parity streams of length 2048; each stream needs
lags -15..+15 only. Correlations are banded matmuls on the PE: contraction
over rows (128/partition group), stationary = 128-wide halo'd window of the
B side (exactly 128 cols -> fast weight load), moving = 98-wide chunk of the
A side; every chunk/parity/row-group accumulates into one (128,98) PSUM tile
per pair since the diagonal offset d = w - i is tiling-invariant. Streams are
zero-padded so all 21 chunks are uniform and DMA rows stay 64B-aligned:
per-core row = [A0|B0|A1|B1], 4 x 2112 = 8448 bytes. Band diagonals are
summed on the host (exact integers scaled by 2^-18).
"""

import numpy as np

N_CORES = 8
B_FULL, L = 4096, 2048
R = 15
B_SH = B_FULL // N_CORES       # 512 batches per core
ROWS = 2 * B_SH                # 1024 rows per core
N_GROUPS = ROWS // 128         # 8
W = 98                         # moving chunk width
WIN = 128                      # stationary window width (exact 128 -> FWL)
HL, HR = 15, 25                # halos on the B parity segments
N_CHUNKS = 21                  # 21 * 98 = 2058 >= L
SEGP = 2112                    # padded segment bytes (mod 64 == 0)
HALF = 2 * SEGP                # 4224: one parity's [A|B] bytes
ROWB = 2 * HALF                # 8448 bytes per row

_CACHE = {}


def _build_program():
    import concourse.bass as bass
    import concourse.tile as tile
    from concourse import mybir

    u8 = mybir.dt.uint8
    u16 = mybir.dt.uint16
    f8 = mybir.dt.float8e4
    f32 = mybir.dt.float32
    Alu = mybir.AluOpType

    nc = bass.Bass()
    ab_d = nc.declare_dram_parameter("ab", [ROWS, ROWB], u8, isOutput=False)
    out_d = nc.declare_dram_parameter("out", [2, WIN, WIN], f32, isOutput=True)

    with tile.TileContext(nc) as tc:
        with (
            tc.tile_pool(name="raw", bufs=6) as raw_pool,
            tc.tile_pool(name="msk", bufs=3) as msk_pool,
            tc.tile_pool(name="acc", bufs=1, space="PSUM") as psum_pool,
        ):
            ps_ab = psum_pool.tile([WIN, W], f32)
            ps_mm = psum_pool.tile([WIN, W], f32)
            ps_w = psum_pool.tile([WIN, W], f32)
            out_sb = msk_pool.tile([WIN, 2, WIN], f32, tag="out", bufs=1)

            # HAM warm-up: dummy matmuls on zeroed scratch during the initial
            # DMA wait, so the real matmuls start at (or near) 2.4 GHz. The
            # tile keeps a full 2112B slot so the pool layout stays one that
            # measures fast LDWEIGHTS (SBUF placement is perf-sensitive).
            warm_t = raw_pool.tile([128, SEGP], u8, tag="warm", bufs=1)
            nc.gpsimd.memset(warm_t[:, : WIN + W], 0)
            for i in range(45):
                nc.tensor.matmul(
                    ps_w[:],
                    warm_t[:, 0:WIN].bitcast(f8),
                    warm_t[:, WIN : WIN + W].bitcast(f8),
                    start=True,
                    stop=True,
                )

            def mm_pair(ab_t, mab_t, g, k, c, pair=None):
                p0 = c * W
                oa = k * HALF + p0
                ob = k * HALF + SEGP + p0
                first = g == 0 and k == 0 and c == 0
                last = g == N_GROUPS - 1 and k == 1 and c == N_CHUNKS - 1
                for src, ps in ((ab_t, ps_ab), (mab_t, ps_mm)):
                    if pair is not None and ps is not pair:
                        continue
                    nc.tensor.matmul(
                        ps[:],
                        src[:, ob : ob + WIN].bitcast(f8),
                        src[:, oa : oa + W].bitcast(f8),
                        start=first,
                        stop=last,
                    )

            def emit_out(j, ps):
                nc.vector.tensor_copy(out_sb[:, j, :W], ps[:])
                (nc.sync if j == 0 else nc.scalar).dma_start(out_d[j], out_sb[:, j])

            for g in range(N_GROUPS):
                rows = slice(g * 128, (g + 1) * 128)
                ab_t = raw_pool.tile([128, ROWB], u8, tag="ab")
                mab_t = msk_pool.tile([128, ROWB], u8, tag="mab")

                for i in range(2):
                    h = slice(i * HALF, (i + 1) * HALF)
                    eng = nc.sync if i == 0 else nc.scalar
                    eng.dma_start(ab_t[:, h], ab_d[rows, h])
                    # strip sign bit to recover masks: m = x & 0x7F
                    nc.vector.tensor_scalar(
                        mab_t[:, h].bitcast(u16), ab_t[:, h].bitcast(u16),
                        0x7F7F, None, op0=Alu.bitwise_and,
                    )

                for k in range(2):
                    if g == N_GROUPS - 1 and k == 1:
                        # final stretch: finish ps_ab's run first so its
                        # copy/out-DMA overlaps the ps_mm run.
                        for c in range(N_CHUNKS):
                            mm_pair(ab_t, mab_t, g, k, c, pair=ps_ab)
                        emit_out(0, ps_ab)
                        for c in range(N_CHUNKS):
                            mm_pair(ab_t, mab_t, g, k, c, pair=ps_mm)
                        emit_out(1, ps_mm)
                    else:
                        for c in range(N_CHUNKS):
                            mm_pair(ab_t, mab_t, g, k, c)

    import bass_rust as _bass_rust

    _bass_rust.move_matmul_waits_to_ldweights(nc.m)
    _bass_rust.generate_event_semaphores(nc)
    return nc


def _get_program():
    if "nc" not in _CACHE:
        _CACHE["nc"] = _build_program()
    return _CACHE["nc"]


def _as_u8(x):
    x = np.asarray(x)
    return x.view(np.uint8) if x.dtype == np.bool_ else x.astype(np.uint8)


def _prep_core(ea, eb, c):
    """Per-core [ROWS, ROWB] u8: rows = (d0,batch) of the core's batch slice,
    row = [A0|B0|A1|B1] with A = parity stream (zero-padded), B = halo'd."""
    sa = ea[:, c * B_SH : (c + 1) * B_SH].reshape(ROWS, L, 2).transpose(0, 2, 1)
    sb = eb[:, c * B_SH : (c + 1) * B_SH].reshape(ROWS, L, 2).transpose(0, 2, 1)
    buf = np.zeros((ROWS, 2, 2, SEGP), np.uint8)
    buf[:, :, 0, :L] = sa
    buf[:, :, 1, :HL] = sb[:, :, -HL:]
    buf[:, :, 1, HL : HL + L] = sb
    buf[:, :, 1, HL + L : HL + L + HR] = sb[:, :, :HR]
    return buf.reshape(ROWS, ROWB)


def kernel(iris_codes_a, mask_codes_a, iris_codes_b, mask_codes_b, _trace=False):
    from concourse.bass_utils import run_bass_kernel_spmd

    nc = _get_program()
    ea = (_as_u8(iris_codes_a) << 7) | _as_u8(mask_codes_a)
    eb = (_as_u8(iris_codes_b) << 7) | _as_u8(mask_codes_b)
    in_maps = [{"ab": _prep_core(ea, eb, c)} for c in range(N_CORES)]
    res = run_bass_kernel_spmd(nc, in_maps, list(range(N_CORES)), trace=_trace)
    _CACHE["last_result"] = res

    acc = np.zeros((2, WIN, WIN), np.float64)
    for r in res.results:
        acc += r["out"].astype(np.float64)

    shifts = np.arange(-R, R + 1)
    ii = np.arange(W)
    cab = np.array([acc[0, ii + HL + s, ii].sum() for s in shifts])
    den = np.array([acc[1, ii + HL + s, ii].sum() for s in shifts])
    cab = np.rint(cab * 2.0**18)
    den = np.rint(den * 2.0**18)
    num = (den - cab) / 2.0
    dist = num.astype(np.float32) / den.astype(np.float32)
    out = np.minimum(np.float32(1.0), dist.min())
    return np.asarray([out], dtype=np.float32)
